# revision 1
# baseline (speedup 1.0000x reference)
"""Trainium2 Bass kernel for nn_AttentionDecoder (single decoder step).

Pure data-parallel across 8 NeuronCores: batch B=128 -> 16 per core, all
weights replicated. Everything below is per-core (shard) unless noted.

Math (per batch row):
  prev_ctx   = prev_alignments @ memory                         [D]
  prev_att   = prev_ctx @ Wa + ba                               [E]
  pre        = relu(relu(x@Wp1+bp1)@Wp2+bp2)                    [H]
  attn_h     = GRU([pre, prev_att], prev_attn_h; Wg,Ug,bg)      [E]
  q          = attn_h @ Wq                                      [A]
  keys       = memory @ Wk                                      [T,A]
  s_t        = v . tanh(q + keys_t)                             [T]
  p          = softmax(s)   (mask is all-ones -> no-op)
  context    = p @ memory                                       [D]
  h1         = GRU([attn_h, context], prev_dec_h1; Wd1,Ud1)     [H]
  h2         = GRU(h1, prev_dec_h2; Wd2,Ud2)                    [H]
  out        = h2 @ Wo + bo                                     [OUT]

Device strategy:
  pass1: stream memory tiles [128t,512d] f32, PE-transpose (f32r) into a
         resident bf16 memT store [d,t], accumulate prev_ctx on PE.
  chain1: all activations live transposed [feat, batch]; matmuls are
         out^T = W.T @ x^T with W chunks stationary.  bf16.
  pass2: keys^T[a,t] = Wk.T @ memT (bf16), ACT tanh(keys + q_bias),
         s[t,1] on PE with tanh stationary / v moving, exp on ACT with
         accum_out denominator, PE ones-matmul replications, context via
         DVE tensor_tensor_reduce against memT.
  chain2: decoder GRUs + output projection, DMA out transposed.
"""

import os
import sys

sys.path.insert(0, "/opt/trn_rl_repo")

import numpy as np
import ml_dtypes

import concourse.bass as bass
import concourse.bacc as bacc
import concourse.tile as tile
import concourse.mybir as mybir
from concourse.bass_utils import run_bass_kernel_spmd

BF_NP = ml_dtypes.bfloat16
F32 = mybir.dt.float32
F32R = mybir.dt.float32r
BF16 = mybir.dt.bfloat16
AF = mybir.ActivationFunctionType
ALU = mybir.AluOpType

NCORES = 8
B, T, D, E, A, H, OUTD = 128, 1024, 512, 512, 512, 256, 400
BL = B // NCORES          # 16 batch rows per core
P = 128
TCH = T // P              # 8 t-chunks of 128
DCH = D // P              # 4
ACH = A // P              # 4
ECH = E // P              # 4
HCH = H // P              # 2
KIN = 512                 # padded input feature dim (400 -> 512)
KOUT = 512                # padded output dim (400 -> 512)
OCH = KOUT // P           # 4


def _emit(nc, dr, cfg):
    """Emit the full decoder step. dr: dict name->AP. cfg: dict with BL/T."""
    bl = cfg["BL"]
    t_dim = cfg["T"]
    tch = t_dim // P
    ts512 = t_dim // 512  # number of 512-wide t chunks

    with tile.TileContext(nc) as tc:
        import contextlib

        ctx = contextlib.ExitStack()
        with ctx:
            # ---------------- long-lived SBUF pools ----------------
            consts = ctx.enter_context(tc.tile_pool(name="consts", bufs=1))
            w512 = ctx.enter_context(tc.tile_pool(name="w512", bufs=14))
            wp2p = ctx.enter_context(tc.tile_pool(name="wp2p", bufs=4))
            bigw = ctx.enter_context(tc.tile_pool(name="bigw", bufs=12))
            memtp = ctx.enter_context(tc.tile_pool(name="memtp", bufs=bl * DCH))
            natp = ctx.enter_context(tc.tile_pool(name="natp", bufs=4))
            tnhp = ctx.enter_context(tc.tile_pool(name="tnhp", bufs=5))
            junkp = ctx.enter_context(tc.tile_pool(name="junkp", bufs=1))
            actp = ctx.enter_context(tc.tile_pool(name="actp", bufs=1))
            svp = ctx.enter_context(tc.tile_pool(name="svp", bufs=3))

            def ct(shape, dt, tag):
                t = consts.tile(shape, dt, tag=tag, name=tag)
                nc.sync.dma_start(t[:], dr[tag][:])
                return t

            # constants / small inputs
            ident = ct([P, P], F32, "ident")
            ones = ct([P, P], F32, "ones")
            onesb = ct([1, P], BF16, "onesb")
            v_sb = ct([P, ACH], BF16, "v_sb")
            xT = ct([P, OCH * bl], BF16, "xT")
            pahT = ct([P, ECH * bl], BF16, "pahT")
            pd1T = ct([P, HCH * bl], BF16, "pd1T")
            pd2T = ct([P, HCH * bl], BF16, "pd2T")
            palT = ct([P, bl * tch], F32, "palT")
            bp1T = ct([P, ECH], F32, "bp1T")
            bp2T = ct([P, HCH], F32, "bp2T")
            baT = ct([P, ECH], F32, "baT")
            bgsT = ct([P, 12], F32, "bgsT")
            bgiT = ct([P, 12], F32, "bgiT")
            bgrT = ct([P, 12], F32, "bgrT")
            bd1sT = ct([P, 6], F32, "bd1sT")
            bd1iT = ct([P, 6], F32, "bd1iT")
            bd1rT = ct([P, 6], F32, "bd1rT")
            bd2sT = ct([P, 6], F32, "bd2sT")
            bd2iT = ct([P, 6], F32, "bd2iT")
            bd2rT = ct([P, 6], F32, "bd2rT")
            boT = ct([P, OCH], F32, "boT")


            def wtiles(name, k, m, pool, nk=None):
                nk = nk if nk is not None else k // P
                out = []
                for kc in range(nk):
                    t = pool.tile([P, m], BF16, tag="w", name=f"{name}_{kc}")
                    nc.sync.dma_start(t[:], dr[name][kc * P : (kc + 1) * P, :])
                    out.append(t)
                return out

            Wp1_sb = wtiles("Wp1", KIN, E, w512)
            Wa_sb = wtiles("Wa", D, E, w512)
            Wq_sb = wtiles("Wq", E, A, w512)
            Wk_sb = wtiles("Wk", D, A, w512)
            Wp2_sb = wtiles("Wp2", E, H, wp2p)
            Wg_sb = wtiles("Wg", H + E, 3 * E, bigw)
            Ug_sb = wtiles("Ug", E, 3 * E, bigw)

            # persistent activation tiles
            qT = actp.tile([P, ACH * bl], F32, tag="qT", name="qT")
            attn_hT = actp.tile([P, ECH * bl], BF16, tag="attn_hT", name="attn_hT")
            ctxT = actp.tile([P, DCH * bl], F32, tag="ctxT", name="ctxT")
            ctxT_bf = actp.tile([P, DCH * bl], BF16, tag="ctxT_bf", name="ctxT_bf")
            pctxT = actp.tile([P, DCH * bl], BF16, tag="pctxT", name="pctxT")
            memT = {}
            for b in range(bl):
                for dc in range(DCH):
                    memT[(b, dc)] = memtp.tile([P, t_dim], BF16, tag="memt", name=f"memT_{b}_{dc}")

            # ================= PASS 1 =================
            with tc.tile_pool(name="p1stg", bufs=4, space="PSUM") as stgp, \
                 tc.tile_pool(name="p1pctx", bufs=4, space="PSUM") as pctxp:
                pctxT_ps = [pctxp.tile([P, bl], F32, tag="pctx", name=f"pctxT_ps{dcx}")
                            for dcx in range(DCH)]
                for b in range(bl):
                    for jg in range(tch // 4):
                        stg = [stgp.tile([P, 512], F32, tag="stg", name=f"stg{b}_{jg}_{dcx}") for dcx in range(DCH)]
                        for j in range(4):
                            tt = jg * 4 + j
                            nat = natp.tile([P, D], F32, tag="nat")
                            nc.sync.dma_start(
                                nat[:], dr["memory"][b, tt * P : (tt + 1) * P, :]
                            )
                            col = b * tch + tt
                            for dc in range(DCH):
                                nc.tensor.matmul(
                                    pctxT_ps[dc][:, b : b + 1],
                                    nat[:, dc * P : (dc + 1) * P],
                                    palT[:, col : col + 1],
                                    start=(tt == 0),
                                    stop=(tt == tch - 1),
                                )
                            for dc in range(DCH):
                                nc.tensor.transpose(
                                    stg[dc][:, j * P : (j + 1) * P],
                                    nat[:, dc * P : (dc + 1) * P],
                                    ident[:],
                                )
                        for dc in range(DCH):
                            nc.vector.tensor_copy(
                                memT[(b, dc)][:, jg * 512 : (jg + 1) * 512],
                                stg[dc][:],
                            )


                for dc in range(DCH):
                    nc.scalar.copy(pctxT[:, dc * bl : (dc + 1) * bl],
                                   pctxT_ps[dc][:])

            if cfg.get("phases", 4) < 2:
                nc.vector.memset(qT[:], 0.0)
                nc.sync.dma_start(dr["out"][:, :4], qT[:bl, :4])
                return
            # ================= CHAIN 1 =================
            with tc.tile_pool(name="cp1", bufs=3, space="PSUM") as cp:
                # prenet layer 1: relu(x@Wp1 + bp1)  -> pre1T [E, b] bf16
                g1 = cp.tile([P, ECH * bl], F32, tag="cps", name="g1")
                for mc in range(ECH):
                    for kc in range(KIN // P):
                        nc.tensor.matmul(
                            g1[:, mc * bl : (mc + 1) * bl],
                            Wp1_sb[kc][:, mc * P : (mc + 1) * P],
                            xT[:, kc * bl : (kc + 1) * bl],
                            start=(kc == 0),
                            stop=(kc == KIN // P - 1),
                        )
                pre1T = svp.tile([P, ECH * bl], BF16, tag="pre1T", name="pre1T")
                for mc in range(ECH):
                    nc.scalar.activation(
                        pre1T[:, mc * bl : (mc + 1) * bl],
                        g1[:, mc * bl : (mc + 1) * bl],
                        AF.Relu,
                        bias=bp1T[:, mc : mc + 1],
                    )
                # prenet layer 2 -> pre2T [H, b] bf16
                g2 = cp.tile([P, HCH * bl], F32, tag="cps", name="g2")
                for mc in range(HCH):
                    for kc in range(ECH):
                        nc.tensor.matmul(
                            g2[:, mc * bl : (mc + 1) * bl],
                            Wp2_sb[kc][:, mc * P : (mc + 1) * P],
                            pre1T[:, kc * bl : (kc + 1) * bl],
                            start=(kc == 0),
                            stop=(kc == ECH - 1),
                        )
                pre2T = svp.tile([P, HCH * bl], BF16, tag="pre2T", name="pre2T")
                for mc in range(HCH):
                    nc.scalar.activation(
                        pre2T[:, mc * bl : (mc + 1) * bl],
                        g2[:, mc * bl : (mc + 1) * bl],
                        AF.Relu,
                        bias=bp2T[:, mc : mc + 1],
                    )
                # prev_attention = prev_ctx @ Wa + ba -> prev_attT [E, b] bf16
                ga = cp.tile([P, ECH * bl], F32, tag="cps", name="ga")
                for mc in range(ECH):
                    for kc in range(DCH):
                        nc.tensor.matmul(
                            ga[:, mc * bl : (mc + 1) * bl],
                            Wa_sb[kc][:, mc * P : (mc + 1) * P],
                            pctxT[:, kc * bl : (kc + 1) * bl],
                            start=(kc == 0),
                            stop=(kc == DCH - 1),
                        )
                prev_attT = svp.tile([P, ECH * bl], BF16, tag="prev_attT", name="prev_attT")
                for mc in range(ECH):
                    nc.scalar.activation(
                        prev_attT[:, mc * bl : (mc + 1) * bl],
                        ga[:, mc * bl : (mc + 1) * bl],
                        AF.Identity,
                        bias=baT[:, mc : mc + 1],
                    )

                # attention GRU
                def gru(cp, n_ch, gi_w, gi_rhs, gr_w, gr_rhs, bsum, bgi, bgr, hT,
                        out_tile):
                    # gi = x@W (+bi later), gr = h@U (+br later); chunk counts:
                    # gates layout m-chunks: [z(n_ch) r(n_ch) c(n_ch)]
                    n3 = 3 * n_ch
                    gi = cp.tile([P, n3 * bl], F32, tag="cps", name="gi")
                    for mc in range(n3):
                        for kc in range(len(gi_w)):
                            nc.tensor.matmul(
                                gi[:, mc * bl : (mc + 1) * bl],
                                gi_w[kc][:, mc * P : (mc + 1) * P],
                                gi_rhs(kc),
                                start=(kc == 0),
                                stop=(kc == len(gi_w) - 1),
                            )
                    gr = cp.tile([P, n3 * bl], F32, tag="cps", name="gr")
                    for mc in range(n3):
                        for kc in range(len(gr_w)):
                            nc.tensor.matmul(
                                gr[:, mc * bl : (mc + 1) * bl],
                                gr_w[kc][:, mc * P : (mc + 1) * P],
                                gr_rhs(kc),
                                start=(kc == 0),
                                stop=(kc == len(gr_w) - 1),
                            )
                    # evict recurrent gates to SBUF with b_r folded in
                    # (DVE cannot read two PSUM operands in one op)
                    grs = svp.tile([P, n3 * bl], F32, tag="grs", name="grs")
                    for mc in range(n3):
                        nc.scalar.activation(
                            grs[:, mc * bl : (mc + 1) * bl],
                            gr[:, mc * bl : (mc + 1) * bl], AF.Identity,
                            bias=bgr[:, mc : mc + 1],
                        )
                    zT = svp.tile([P, n_ch * bl], BF16, tag="zT", name="zT")
                    rT = svp.tile([P, n_ch * bl], BF16, tag="rT", name="rT")
                    cT = svp.tile([P, n_ch * bl], BF16, tag="cT", name="cT")
                    tmp = svp.tile([P, n_ch * bl], F32, tag="gtmp", name="gtmp")
                    for mc in range(n_ch):
                        sl = slice(mc * bl, (mc + 1) * bl)
                        nc.vector.tensor_add(tmp[:, sl], gi[:, sl], grs[:, sl])
                        nc.scalar.activation(
                            zT[:, sl], tmp[:, sl], AF.Sigmoid,
                            bias=bgi[:, mc : mc + 1],
                        )
                    for mc in range(n_ch):
                        sl = slice(mc * bl, (mc + 1) * bl)
                        slg = slice((n_ch + mc) * bl, (n_ch + mc + 1) * bl)
                        nc.vector.tensor_add(tmp[:, sl], gi[:, slg], grs[:, slg])
                        nc.scalar.activation(
                            rT[:, sl], tmp[:, sl], AF.Sigmoid,
                            bias=bgi[:, n_ch + mc : n_ch + mc + 1],
                        )
                    grc = svp.tile([P, n_ch * bl], F32, tag="grc", name="grc")
                    for mc in range(n_ch):
                        sl = slice(mc * bl, (mc + 1) * bl)
                        slg = slice((2 * n_ch + mc) * bl, (2 * n_ch + mc + 1) * bl)
                        nc.vector.tensor_mul(grc[:, sl], rT[:, sl], grs[:, slg])
                        nc.vector.tensor_add(grc[:, sl], gi[:, slg], grc[:, sl])
                        nc.scalar.activation(
                            cT[:, sl], grc[:, sl], AF.Tanh,
                            bias=bgi[:, 2 * n_ch + mc : 2 * n_ch + mc + 1],
                        )
                    # h' = c + z*(h - c)
                    dT = svp.tile([P, n_ch * bl], BF16, tag="dT", name="dT")
                    nc.vector.tensor_tensor(dT[:], hT[:], cT[:], ALU.subtract)
                    nc.vector.tensor_mul(dT[:], zT[:], dT[:])
                    nc.vector.tensor_add(out_tile[:], cT[:], dT[:])

                def gi_rhs_attn(kc):
                    if kc < HCH:
                        return pre2T[:, kc * bl : (kc + 1) * bl]
                    return prev_attT[:, (kc - HCH) * bl : (kc - HCH + 1) * bl]

                gru(cp, ECH, Wg_sb, gi_rhs_attn, Ug_sb,
                    lambda kc: pahT[:, kc * bl : (kc + 1) * bl],
                    bgsT, bgiT, bgrT, pahT, attn_hT)

                # q = attn_h @ Wq  -> qT [A, b] f32
                gq = cp.tile([P, ACH * bl], F32, tag="cps", name="gq")
                for mc in range(ACH):
                    for kc in range(ECH):
                        nc.tensor.matmul(
                            gq[:, mc * bl : (mc + 1) * bl],
                            Wq_sb[kc][:, mc * P : (mc + 1) * P],
                            attn_hT[:, kc * bl : (kc + 1) * bl],
                            start=(kc == 0),
                            stop=(kc == ECH - 1),
                        )
                nc.scalar.copy(qT[:], gq[:])

            if cfg.get("phases", 4) < 3:
                nc.vector.memset(qT[:], 0.0)
                nc.sync.dma_start(dr["out"][:, :4], qT[:bl, :4])
                return
            # ================= PASS 2 =================
            with tc.tile_pool(name="kp", bufs=2, space="PSUM") as kp, \
                 tc.tile_pool(name="sp", bufs=1, space="PSUM") as sp, \
                 tc.tile_pool(name="zp", bufs=1, space="PSUM") as zp, \
                 tc.tile_pool(name="cxp", bufs=4, space="PSUM") as cxp:
                ctxT_ps = [cxp.tile([P, bl], F32, tag="cx", name=f"ctxT_ps{dcx}")
                           for dcx in range(DCH)]
                for b in range(bl):
                    s_ps = sp.tile([P, tch], F32, tag="s", name=f"s{b}")
                    for tci in range(ts512):
                        ths = []
                        for at in range(ACH):
                            kt = kp.tile([P, 512], F32, tag="k", name=f"kt{b}_{tci}_{at}")
                            for dc in range(DCH):
                                nc.tensor.matmul(
                                    kt[:],
                                    Wk_sb[dc][:, at * P : (at + 1) * P],
                                    memT[(b, dc)][:, tci * 512 : (tci + 1) * 512],
                                    start=(dc == 0),
                                    stop=(dc == DCH - 1),
                                )
                            th = tnhp.tile([P, 512], BF16, tag="tanh", name=f"th{b}_{tci}_{at}")
                            nc.scalar.activation(
                                th[:], kt[:], AF.Tanh,
                                bias=qT[:, at * bl + b : at * bl + b + 1],
                            )
                            ths.append(th)
                        # one accumulation group (column) at a time per bank
                        for tsub in range(4):
                            ttile = tci * 4 + tsub
                            for at in range(ACH):
                                nc.tensor.matmul(
                                    s_ps[:, ttile : ttile + 1],
                                    ths[at][:, tsub * P : (tsub + 1) * P],
                                    v_sb[:, at : at + 1],
                                    start=(at == 0),
                                    stop=(at == ACH - 1),
                                )
                    # softmax in column layout (no max-sub; scores are small)
                    exp_sb = svp.tile([P, tch], F32, tag="exp", name=f"exp{b}")
                    nc.scalar.activation(exp_sb[:], s_ps[:], AF.Exp)
                    r_col = svp.tile([P, 1], F32, tag="r", name=f"r{b}")
                    nc.vector.tensor_reduce(r_col[:], exp_sb[:],
                                            mybir.AxisListType.X, ALU.add)
                    z_ps = zp.tile([P, 1], F32, tag="z", name=f"z{b}")
                    nc.tensor.matmul(z_ps[:], ones[:], r_col[:])
                    invz = svp.tile([P, 1], F32, tag="invz", name=f"invz{b}")
                    nc.vector.reciprocal(invz[:], z_ps[:])
                    nc.vector.tensor_scalar_mul(exp_sb[:], exp_sb[:], invz[:])
                    # context^T: re-stream natural memory tiles, accumulate
                    # ctxT[:, dc*bl+b] over t-chunks (same class as prev_ctx)
                    for tt in range(tch):
                        nat2 = natp.tile([P, D], F32, tag="nat", name=f"nat2_{b}_{tt}")
                        nc.sync.dma_start(
                            nat2[:], dr["memory"][b, tt * P : (tt + 1) * P, :]
                        )
                        for dc in range(DCH):
                            nc.tensor.matmul(
                                ctxT_ps[dc][:, b : b + 1],
                                nat2[:, dc * P : (dc + 1) * P],
                                exp_sb[:, tt : tt + 1],
                                start=(tt == 0),
                                stop=(tt == tch - 1),
                            )
                for dc in range(DCH):
                    nc.scalar.copy(ctxT[:, dc * bl : (dc + 1) * bl],
                                   ctxT_ps[dc][:])
                nc.vector.tensor_copy(ctxT_bf[:], ctxT[:])

            if cfg.get("phases", 4) < 4:
                nc.vector.memset(qT[:], 0.0)
                nc.sync.dma_start(dr["out"][:, :4], qT[:bl, :4])
                return
            # ================= CHAIN 2 =================
            Wd1_sb = wtiles("Wd1", E + D, 3 * H, bigw)
            Ud1_sb = wtiles("Ud1", H, 3 * H, bigw)
            Wd2_sb = wtiles("Wd2", H, 3 * H, bigw)
            Ud2_sb = wtiles("Ud2", H, 3 * H, bigw)
            Wo_sb = wtiles("Wo", H, KOUT, w512)

            with tc.tile_pool(name="cp2", bufs=3, space="PSUM") as cp:
                h1T = svp.tile([P, HCH * bl], BF16, tag="h1T", name="h1T")
                h2T = svp.tile([P, HCH * bl], BF16, tag="h2T", name="h2T")

                def gi_rhs_d1(kc):
                    if kc < ECH:
                        return attn_hT[:, kc * bl : (kc + 1) * bl]
                    return ctxT_bf[:, (kc - ECH) * bl : (kc - ECH + 1) * bl]

                gru(cp, HCH, Wd1_sb, gi_rhs_d1, Ud1_sb,
                    lambda kc: pd1T[:, kc * bl : (kc + 1) * bl],
                    bd1sT, bd1iT, bd1rT, pd1T, h1T)
                gru(cp, HCH, Wd2_sb,
                    lambda kc: h1T[:, kc * bl : (kc + 1) * bl],
                    Ud2_sb,
                    lambda kc: pd2T[:, kc * bl : (kc + 1) * bl],
                    bd2sT, bd2iT, bd2rT, pd2T, h2T)

                # out^T = Wo.T @ h2T + bo
                go = cp.tile([P, OCH * bl], F32, tag="cps", name="go")
                for mc in range(OCH):
                    for kc in range(HCH):
                        nc.tensor.matmul(
                            go[:, mc * bl : (mc + 1) * bl],
                            Wo_sb[kc][:, mc * P : (mc + 1) * P],
                            h2T[:, kc * bl : (kc + 1) * bl],
                            start=(kc == 0),
                            stop=(kc == HCH - 1),
                        )
                outT = svp.tile([P, OCH * bl], F32, tag="outT", name="outT")
                for mc in range(OCH):
                    nc.scalar.activation(
                        outT[:, mc * bl : (mc + 1) * bl],
                        go[:, mc * bl : (mc + 1) * bl],
                        AF.Identity,
                        bias=boT[:, mc : mc + 1],
                    )
                for mc in range(OCH):
                    sz = min(P, OUTD - mc * P)
                    if sz <= 0:
                        break
                    nc.sync.dma_start(
                        dr["out"][:, mc * P : mc * P + sz].rearrange("b o -> o b"),
                        outT[:sz, mc * bl : (mc + 1) * bl],
                    )


def build(cfg=None):
    cfg = cfg or {"BL": BL, "T": T}
    nc = bacc.Bacc("TRN2", target_bir_lowering=False, debug=False,
                   num_devices=NCORES)
    bl, t_dim = cfg["BL"], cfg["T"]
    tch = t_dim // P
    dr = {}

    def din(name, shape, dt=F32):
        dr[name] = nc.dram_tensor(name, list(shape), dt, kind="ExternalInput").ap()

    din("memory", [bl, t_dim, D])
    din("ident", [P, P])
    din("ones", [P, P])
    din("onesb", [1, P], BF16)
    din("v_sb", [P, ACH], BF16)
    din("xT", [P, OCH * bl], BF16)
    din("pahT", [P, ECH * bl], BF16)
    din("pd1T", [P, HCH * bl], BF16)
    din("pd2T", [P, HCH * bl], BF16)
    din("palT", [P, bl * tch])
    for nm, sh in [("bp1T", [P, ECH]), ("bp2T", [P, HCH]), ("baT", [P, ECH]),
                   ("bgsT", [P, 12]), ("bgiT", [P, 12]), ("bgrT", [P, 12]),
                   ("bd1sT", [P, 6]), ("bd1iT", [P, 6]), ("bd1rT", [P, 6]),
                   ("bd2sT", [P, 6]), ("bd2iT", [P, 6]), ("bd2rT", [P, 6]),
                   ("boT", [P, OCH])]:
        din(nm, sh)
    for nm, sh in [("Wp1", [KIN, E]), ("Wp2", [E, H]), ("Wa", [D, E]),
                   ("Wq", [E, A]), ("Wk", [D, A]), ("Wg", [H + E, 3 * E]),
                   ("Ug", [E, 3 * E]), ("Wd1", [E + D, 3 * H]),
                   ("Ud1", [H, 3 * H]), ("Wd2", [H, 3 * H]),
                   ("Ud2", [H, 3 * H]), ("Wo", [H, KOUT])]:
        din(nm, sh, BF16)
    dr["out"] = nc.dram_tensor("out", [bl, OUTD], F32, kind="ExternalOutput").ap()

    _emit(nc, dr, cfg)
    nc.compile()
    return nc


# ---------------- host-side data prep ----------------

def _chunkT(mat, pad_rows=None):
    """[b, F] -> transposed chunk layout [128, nch*b] (col = chunk*b + batch)."""
    a = np.asarray(mat, np.float32).T  # [F, b]
    f, b = a.shape
    if pad_rows and f < pad_rows:
        a = np.concatenate([a, np.zeros((pad_rows - f, b), np.float32)], 0)
    f = a.shape[0]
    nch = f // P
    return np.ascontiguousarray(
        a.reshape(nch, P, b).transpose(1, 0, 2).reshape(P, nch * b)
    )


def _biasT(vec, pad_to=None):
    a = np.asarray(vec, np.float32)
    if pad_to and a.shape[0] < pad_to:
        a = np.concatenate([a, np.zeros(pad_to - a.shape[0], np.float32)])
    nch = a.shape[0] // P
    return np.ascontiguousarray(a.reshape(nch, P).T)


def _prep_shared(inp):
    """Weights + constants shared by all cores."""
    bf = lambda x, pad=None: np.ascontiguousarray(
        (np.concatenate([np.asarray(x, np.float32),
                         np.zeros((pad[0] - x.shape[0], x.shape[1]), np.float32)], 0)
         if pad and x.shape[0] < pad[0] else
         np.concatenate([np.asarray(x, np.float32),
                         np.zeros((x.shape[0], pad[1] - x.shape[1]), np.float32)], 1)
         if pad and x.shape[1] < pad[1] else np.asarray(x, np.float32)
         ).astype(BF_NP))

    sh = {
        "ident": np.eye(P, dtype=np.float32),
        "ones": np.ones((P, P), np.float32),
        "onesb": np.ones((1, P), BF_NP),
        "v_sb": np.ascontiguousarray(
            np.asarray(inp["v_attn"], np.float32).reshape(ACH, P).T.astype(BF_NP)),
        "Wp1": bf(inp["Wp1"], pad=(KIN, E)),
        "Wp2": bf(inp["Wp2"]),
        "Wa": bf(inp["Wa"]),
        "Wq": bf(inp["Wq"]),
        "Wk": bf(inp["Wk"]),
        "Wg": bf(inp["Wg"]),
        "Ug": bf(inp["Ug"]),
        "Wd1": bf(inp["Wd1"]),
        "Ud1": bf(inp["Ud1"]),
        "Wd2": bf(inp["Wd2"]),
        "Ud2": bf(inp["Ud2"]),
        "Wo": bf(inp["Wo"], pad=(H, KOUT)),
        "bp1T": _biasT(inp["bp1"]),
        "bp2T": _biasT(inp["bp2"]),
        "baT": _biasT(inp["ba"]),
        "bgsT": _biasT(np.asarray(inp["bg_i"]) + np.asarray(inp["bg_r"])),
        "bgiT": _biasT(inp["bg_i"]),
        "bgrT": _biasT(inp["bg_r"]),
        "bd1sT": _biasT(np.asarray(inp["bd1_i"]) + np.asarray(inp["bd1_r"])),
        "bd1iT": _biasT(inp["bd1_i"]),
        "bd1rT": _biasT(inp["bd1_r"]),
        "bd2sT": _biasT(np.asarray(inp["bd2_i"]) + np.asarray(inp["bd2_r"])),
        "bd2iT": _biasT(inp["bd2_i"]),
        "bd2rT": _biasT(inp["bd2_r"]),
        "boT": _biasT(inp["bo"], pad_to=KOUT),
    }
    return sh


def _prep_core(inp, c, bl=BL, t_dim=T):
    tch = t_dim // P
    sl = slice(c * bl, (c + 1) * bl)
    mem = np.ascontiguousarray(np.asarray(inp["memory"], np.float32)[sl])
    pal = np.asarray(inp["prev_alignments"], np.float32)[sl]  # [bl, t]
    palT = np.ascontiguousarray(
        pal.reshape(bl, tch, P).transpose(2, 0, 1).reshape(P, bl * tch))
    return {
        "memory": mem,
        "xT": _chunkT(np.asarray(inp["inputs"], np.float32)[sl],
                      pad_rows=KIN).astype(BF_NP),
        "pahT": _chunkT(np.asarray(inp["prev_attn_h"], np.float32)[sl]).astype(BF_NP),
        "pd1T": _chunkT(np.asarray(inp["prev_dec_h1"], np.float32)[sl]).astype(BF_NP),
        "pd2T": _chunkT(np.asarray(inp["prev_dec_h2"], np.float32)[sl]).astype(BF_NP),
        "palT": palT,
    }


_NC_CACHE = {}


def _get_nc():
    if "nc" not in _NC_CACHE:
        _NC_CACHE["nc"] = build()
    return _NC_CACHE["nc"]


def _run(inputs, **kw):
    nc = _get_nc()
    sh = _prep_shared(inputs)
    in_maps = [dict(sh, **_prep_core(inputs, c)) for c in range(NCORES)]
    res = run_bass_kernel_spmd(nc, in_maps, core_ids=list(range(NCORES)), **kw)
    out = np.concatenate([res.results[c]["out"] for c in range(NCORES)], 0)
    return out.reshape(B, 1, OUTD).astype(np.float32), res


def kernel(**inputs):
    out, _ = _run(inputs)
    return out


def _install_ntff_hook():
    """Register the axon NTFF profiling hook (missing antenv.axon_hooks)."""
    import contextlib
    import ctypes
    import types

    if "antenv.axon_hooks" in sys.modules:
        return
    lib = ctypes.CDLL("/opt/axon/libaxon_pjrt.so")
    if not hasattr(lib, "axon_start_nrt_profile"):
        return
    lib.axon_start_nrt_profile.argtypes = [
        ctypes.POINTER(ctypes.c_int64), ctypes.c_size_t]
    lib.axon_start_nrt_profile.restype = ctypes.c_int64
    lib.axon_stop_nrt_profile.argtypes = [ctypes.c_char_p]
    lib.axon_stop_nrt_profile.restype = ctypes.c_int64

    @contextlib.contextmanager
    def _hook(output_dir, device_ids):
        import jax

        jax.devices()
        if device_ids:
            ids = (ctypes.c_int64 * len(device_ids))(*device_ids)
            rc = lib.axon_start_nrt_profile(ids, len(device_ids))
        else:
            rc = lib.axon_start_nrt_profile(None, 0)
        if rc != 0:
            raise RuntimeError(f"axon_start_nrt_profile rc={rc}")
        try:
            yield
        finally:
            n = lib.axon_stop_nrt_profile(str(output_dir).encode())
            print(f"ntff profile: {n} file(s) written to {output_dir}")

    mod = types.ModuleType("antenv.axon_hooks")
    mod.get_axon_ntff_profile_hook = lambda: _hook
    mod.set_axon_ntff_profile_hook = lambda h: None
    sys.modules["antenv.axon_hooks"] = mod
    import antenv

    antenv.axon_hooks = mod


def kernel_traced(**inputs):
    """Dev helper: returns (output, BassKernelResults with exec_time_ns)."""
    _install_ntff_hook()
    return _run(inputs, trace=True)



# revision 6
# speedup vs baseline: 2.9223x; 2.9223x over previous
"""Trainium2 Bass kernel for nn_AttentionDecoder (single decoder step).

Pure data-parallel across 8 NeuronCores: batch B=128 -> 16 per core, all
weights replicated. Everything below is per-core (shard) unless noted.

Math (per batch row):
  prev_ctx   = prev_alignments @ memory                         [D]
  prev_att   = prev_ctx @ Wa + ba                               [E]
  pre        = relu(relu(x@Wp1+bp1)@Wp2+bp2)                    [H]
  attn_h     = GRU([pre, prev_att], prev_attn_h; Wg,Ug,bg)      [E]
  q          = attn_h @ Wq                                      [A]
  keys       = memory @ Wk                                      [T,A]
  s_t        = v . tanh(q + keys_t)                             [T]
  p          = softmax(s)   (mask is all-ones -> no-op)
  context    = p @ memory                                       [D]
  h1         = GRU([attn_h, context], prev_dec_h1; Wd1,Ud1)     [H]
  h2         = GRU(h1, prev_dec_h2; Wd2,Ud2)                    [H]
  out        = h2 @ Wo + bo                                     [OUT]

Device strategy (v2 — no on-device transpose):
  Host pre-transposes memory to bf16 memT [d,t] tiles (layout work only,
  no math). Phase A streams memT resident in SBUF; prev_ctx runs on DVE
  as affine_mul_reduce against a PE-broadcast of the alignment row.
  Chain1 (prenet + attention GRU + q) runs on PE with z/r gates fused
  into one PSUM accumulation (W and U matmuls share the bank).
  Phase C per batch row: keys^T = Wk.T @ memT (bf16, Wk stationary),
  ACT tanh(keys + q) PSUM->SBUF, s = v^T th with v stationary (PSUM
  row 0), ACT exp with accum_out denominator, DVE normalize, PE
  ones-outer-product broadcast of p, DVE affine_mul_reduce context.
  Chain2 (decoder GRUs + output projection) then DMAs out transposed.
"""

import os
import sys

sys.path.insert(0, "/opt/trn_rl_repo")

import numpy as np
import ml_dtypes

import concourse.bass as bass
import concourse.bacc as bacc
import concourse.tile as tile
import concourse.mybir as mybir
from concourse.bass_utils import run_bass_kernel_spmd

BF_NP = ml_dtypes.bfloat16
F32 = mybir.dt.float32
BF16 = mybir.dt.bfloat16
AF = mybir.ActivationFunctionType
ALU = mybir.AluOpType

NCORES = 8
B, T, D, E, A, H, OUTD = 128, 1024, 512, 512, 512, 256, 400
BL = B // NCORES          # 16 batch rows per core
P = 128
TCH = T // P              # 8
DCH = D // P              # 4
ACH = A // P              # 4
ECH = E // P              # 4
HCH = H // P              # 2
KIN = 512                 # padded input feature dim (400 -> 512)
KOUT = 512                # padded output dim (400 -> 512)
OCH = KOUT // P           # 4


def _emit(nc, dr, cfg):
    bl = cfg["BL"]
    t_dim = cfg["T"]

    with tile.TileContext(nc) as tc:
        import contextlib

        ctx = contextlib.ExitStack()
        with ctx:
            # ---------------- persistent SBUF pools ----------------
            consts = ctx.enter_context(tc.tile_pool(name="consts", bufs=1))
            wsm = ctx.enter_context(tc.tile_pool(name="wsm", bufs=1))
            memtp = ctx.enter_context(tc.tile_pool(name="memtp", bufs=bl * DCH))
            actp = ctx.enter_context(tc.tile_pool(name="actp", bufs=1))
            svp = ctx.enter_context(tc.tile_pool(name="svp", bufs=2))

            def ct(shape, dt, tag):
                t = consts.tile(shape, dt, tag=tag, name=tag)
                nc.gpsimd.dma_start(t[:], dr[tag][:])
                return t

            onesb = ct([1, P], BF16, "onesb")
            v_sb = ct([P, ACH], BF16, "v_sb")
            xT = ct([P, OCH * bl], BF16, "xT")
            pahT = ct([P, ECH * bl], BF16, "pahT")
            pd1T = ct([P, HCH * bl], BF16, "pd1T")
            pd2T = ct([P, HCH * bl], BF16, "pd2T")
            bp1T = ct([P, ECH], F32, "bp1T")
            bp2T = ct([P, HCH], F32, "bp2T")
            baT = ct([P, ECH], F32, "baT")
            bgsT = ct([P, 12], F32, "bgsT")
            bgiT = ct([P, 12], F32, "bgiT")
            bgrT = ct([P, 12], F32, "bgrT")
            bd1sT = ct([P, 6], F32, "bd1sT")
            bd1iT = ct([P, 6], F32, "bd1iT")
            bd1rT = ct([P, 6], F32, "bd1rT")
            bd2sT = ct([P, 6], F32, "bd2sT")
            bd2iT = ct([P, 6], F32, "bd2iT")
            bd2rT = ct([P, 6], F32, "bd2rT")
            boT = ct([P, OCH], F32, "boT")

            def wtiles(name, k, pool, queue=None):
                q = queue or nc.gpsimd
                out = []
                for kc in range(k // P):
                    t = pool.tile([P, dr[name].shape[1]], BF16, tag=f"w_{name}_{kc}",
                                  name=f"{name}_{kc}")
                    q.dma_start(t[:], dr[name][kc * P: (kc + 1) * P, :])
                    out.append(t)
                return out

            Wp1_sb = wtiles("Wp1", KIN, wsm)
            Wp2_sb = wtiles("Wp2", E, wsm)
            Wa_sb = wtiles("Wa", D, wsm)
            Wq_sb = wtiles("Wq", E, wsm)
            Wk_sb = wtiles("Wk", D, wsm)

            # persistent activation tiles
            qT = actp.tile([P, ACH * bl], F32, tag="qT", name="qT")
            attn_hT = actp.tile([P, ECH * bl], BF16, tag="attn_hT", name="attn_hT")
            pctxF = actp.tile([P, DCH * bl], F32, tag="pctxF", name="pctxF")
            pctxT = actp.tile([P, DCH * bl], BF16, tag="pctxT", name="pctxT")
            ctxF = actp.tile([P, DCH * bl], F32, tag="ctxF", name="ctxF")
            ctxT_bf = actp.tile([P, DCH * bl], BF16, tag="ctxT_bf", name="ctxT_bf")

            memT = {}
            for b in range(bl):
                for dc in range(DCH):
                    memT[(b, dc)] = memtp.tile([P, t_dim], BF16, tag="memt",
                                               name=f"memT_{b}_{dc}")

            # ---------- fused-z/r GRU ----------
            def gru(cps, n_ch, W_sb, U_sb, gi_rhs, gr_rhs, bsum, bgi, bgr, hT,
                    out_tile):
                nW, nU = len(W_sb), len(U_sb)
                n2 = 2 * n_ch
                # z/r gates: W and U accumulate into one PSUM group
                zr = cps.tile([P, n2 * bl], F32, tag="cps", name="zr")
                for mc in range(n2):
                    for kc in range(nW + nU):
                        w = (W_sb[kc][:, mc * P: (mc + 1) * P] if kc < nW
                             else U_sb[kc - nW][:, mc * P: (mc + 1) * P])
                        rhs = gi_rhs(kc) if kc < nW else gr_rhs(kc - nW)
                        nc.tensor.matmul(
                            zr[:, mc * bl: (mc + 1) * bl], w, rhs,
                            start=(kc == 0), stop=(kc == nW + nU - 1))
                # candidate gate: keep input/recurrent parts separate
                ci = cps.tile([P, n_ch * bl], F32, tag="cps", name="ci")
                cr = cps.tile([P, n_ch * bl], F32, tag="cps", name="cr")
                for mc in range(n_ch):
                    mg = n2 + mc
                    for kc in range(nW):
                        nc.tensor.matmul(
                            ci[:, mc * bl: (mc + 1) * bl],
                            W_sb[kc][:, mg * P: (mg + 1) * P], gi_rhs(kc),
                            start=(kc == 0), stop=(kc == nW - 1))
                    for kc in range(nU):
                        nc.tensor.matmul(
                            cr[:, mc * bl: (mc + 1) * bl],
                            U_sb[kc][:, mg * P: (mg + 1) * P], gr_rhs(kc),
                            start=(kc == 0), stop=(kc == nU - 1))
                zT = svp.tile([P, n_ch * bl], BF16, tag="zT", name="zT")
                rT = svp.tile([P, n_ch * bl], BF16, tag="rT", name="rT")
                for mc in range(n_ch):
                    sl = slice(mc * bl, (mc + 1) * bl)
                    nc.scalar.activation(zT[:, sl], zr[:, sl], AF.Sigmoid,
                                         bias=bsum[:, mc: mc + 1])
                for mc in range(n_ch):
                    sl = slice(mc * bl, (mc + 1) * bl)
                    slr = slice((n_ch + mc) * bl, (n_ch + mc + 1) * bl)
                    nc.scalar.activation(rT[:, sl], zr[:, slr], AF.Sigmoid,
                                         bias=bsum[:, n_ch + mc: n_ch + mc + 1])
                grs = svp.tile([P, n_ch * bl], F32, tag="grs", name="grs")
                for mc in range(n_ch):
                    sl = slice(mc * bl, (mc + 1) * bl)
                    nc.scalar.activation(grs[:, sl], cr[:, sl], AF.Identity,
                                         bias=bgr[:, n2 + mc: n2 + mc + 1])
                tmp = svp.tile([P, n_ch * bl], F32, tag="gtmp", name="gtmp")
                nc.vector.tensor_mul(tmp[:], rT[:], grs[:])
                tmp2 = svp.tile([P, n_ch * bl], F32, tag="gtmp2", name="gtmp2")
                nc.vector.tensor_add(tmp2[:], ci[:], tmp[:])
                cT = svp.tile([P, n_ch * bl], BF16, tag="cT", name="cT")
                for mc in range(n_ch):
                    sl = slice(mc * bl, (mc + 1) * bl)
                    nc.scalar.activation(cT[:, sl], tmp2[:, sl], AF.Tanh,
                                         bias=bgi[:, n2 + mc: n2 + mc + 1])
                # h' = c + z*(h - c)
                dT = svp.tile([P, n_ch * bl], BF16, tag="dT", name="dT")
                nc.vector.tensor_tensor(dT[:], hT[:], cT[:], ALU.subtract)
                nc.vector.tensor_mul(dT[:], zT[:], dT[:])
                nc.vector.tensor_add(out_tile[:], cT[:], dT[:])

            # ================= PHASE A + CHAIN 1 =================
            with tc.tile_pool(name="wgug", bufs=1) as wgug:
                Wg_sb = wtiles("Wg", H + E, wgug)
                Ug_sb = wtiles("Ug", E, wgug)

                with tc.tile_pool(name="pArows", bufs=2) as pArows, \
                     tc.tile_pool(name="pAb", bufs=2) as pAb, \
                     tc.tile_pool(name="pAscr", bufs=1) as pAscr, \
                     tc.tile_pool(name="pAps", bufs=2, space="PSUM") as pAps, \
                     tc.tile_pool(name="cp1", bufs=3, space="PSUM") as cp1:
                    # prenet layer 1 on PE while the stream starts
                    g1 = cp1.tile([P, ECH * bl], F32, tag="cps", name="g1")
                    for mc in range(ECH):
                        for kc in range(KIN // P):
                            nc.tensor.matmul(
                                g1[:, mc * bl: (mc + 1) * bl],
                                Wp1_sb[kc][:, mc * P: (mc + 1) * P],
                                xT[:, kc * bl: (kc + 1) * bl],
                                start=(kc == 0), stop=(kc == KIN // P - 1))
                    pre1T = svp.tile([P, ECH * bl], BF16, tag="pre1T", name="pre1T")
                    for mc in range(ECH):
                        nc.scalar.activation(
                            pre1T[:, mc * bl: (mc + 1) * bl],
                            g1[:, mc * bl: (mc + 1) * bl], AF.Relu,
                            bias=bp1T[:, mc: mc + 1])
                    g2 = cp1.tile([P, HCH * bl], F32, tag="cps", name="g2")
                    for mc in range(HCH):
                        for kc in range(ECH):
                            nc.tensor.matmul(
                                g2[:, mc * bl: (mc + 1) * bl],
                                Wp2_sb[kc][:, mc * P: (mc + 1) * P],
                                pre1T[:, kc * bl: (kc + 1) * bl],
                                start=(kc == 0), stop=(kc == ECH - 1))
                    pre2T = svp.tile([P, HCH * bl], BF16, tag="pre2T", name="pre2T")
                    for mc in range(HCH):
                        nc.scalar.activation(
                            pre2T[:, mc * bl: (mc + 1) * bl],
                            g2[:, mc * bl: (mc + 1) * bl], AF.Relu,
                            bias=bp2T[:, mc: mc + 1])

                    # stream memT; prev_ctx per (b, dc) on DVE
                    for b in range(bl):
                        prow = pArows.tile([1, t_dim], BF16, tag="palrow",
                                           name=f"palrow{b}")
                        nc.sync.dma_start(prow[:], dr["palr"][b: b + 1, :])
                        for dc in range(DCH):
                            nc.sync.dma_start(memT[(b, dc)][:],
                                              dr["memT"][b, dc])
                        pbc = pAps.tile([P, t_dim], F32, tag="pbc",
                                        name=f"pbc{b}")
                        nc.tensor.matmul(pbc[:, 0:512], onesb[:], prow[:, 0:512],
                                         start=True, stop=True)
                        nc.tensor.matmul(pbc[:, 512:1024], onesb[:],
                                         prow[:, 512:1024], start=True, stop=True)
                        palB = pAb.tile([P, t_dim], BF16, tag="palB",
                                        name=f"palB{b}")
                        nc.scalar.copy(palB[:], pbc[:])
                        for dc in range(DCH):
                            scr = pAscr.tile([P, t_dim], BF16, tag="pAscr",
                                             name=f"pAscr{b}_{dc}")
                            nc.vector.affine_mul_reduce(
                                out=scr[:],
                                accum_out=pctxF[:, dc * bl + b: dc * bl + b + 1],
                                in0=memT[(b, dc)][:], in1=palB[:],
                                scale=1.0, bias=0.0)
                    nc.vector.tensor_copy(pctxT[:], pctxF[:])

                    if cfg.get("phases", 4) < 2:
                        nc.sync.dma_start(dr["out"][:, :4], pctxF[:bl, :4])
                        return

                    # ---------------- CHAIN 1 ----------------
                    # prev_attention = prev_ctx @ Wa + ba
                    ga = cp1.tile([P, ECH * bl], F32, tag="cps", name="ga")
                    for mc in range(ECH):
                        for kc in range(DCH):
                            nc.tensor.matmul(
                                ga[:, mc * bl: (mc + 1) * bl],
                                Wa_sb[kc][:, mc * P: (mc + 1) * P],
                                pctxT[:, kc * bl: (kc + 1) * bl],
                                start=(kc == 0), stop=(kc == DCH - 1))
                    prev_attT = svp.tile([P, ECH * bl], BF16, tag="prev_attT",
                                         name="prev_attT")
                    for mc in range(ECH):
                        nc.scalar.activation(
                            prev_attT[:, mc * bl: (mc + 1) * bl],
                            ga[:, mc * bl: (mc + 1) * bl], AF.Identity,
                            bias=baT[:, mc: mc + 1])

                    def gi_rhs_attn(kc):
                        if kc < HCH:
                            return pre2T[:, kc * bl: (kc + 1) * bl]
                        return prev_attT[:, (kc - HCH) * bl: (kc - HCH + 1) * bl]

                    gru(cp1, ECH, Wg_sb, Ug_sb, gi_rhs_attn,
                        lambda kc: pahT[:, kc * bl: (kc + 1) * bl],
                        bgsT, bgiT, bgrT, pahT, attn_hT)

                    gq = cp1.tile([P, ACH * bl], F32, tag="cps", name="gq")
                    for mc in range(ACH):
                        for kc in range(ECH):
                            nc.tensor.matmul(
                                gq[:, mc * bl: (mc + 1) * bl],
                                Wq_sb[kc][:, mc * P: (mc + 1) * P],
                                attn_hT[:, kc * bl: (kc + 1) * bl],
                                start=(kc == 0), stop=(kc == ECH - 1))
                    nc.scalar.copy(qT[:], gq[:])

            if cfg.get("phases", 4) < 3:
                nc.sync.dma_start(dr["out"][:, :4], qT[:bl, :4])
                return

            # ================= PHASE C =================
            with tc.tile_pool(name="thp", bufs=8) as thp, \
                 tc.tile_pool(name="pbp", bufs=2) as pbp, \
                 tc.tile_pool(name="rows", bufs=2) as rows, \
                 tc.tile_pool(name="scrC", bufs=1) as scrC, \
                 tc.tile_pool(name="c2w", bufs=1) as c2w:
                # chain-2 weights arrive during phase C (gpsimd queue)
                Wd1_sb = wtiles("Wd1", E + D, c2w)
                Ud1_sb = wtiles("Ud1", H, c2w)
                Wd2_sb = wtiles("Wd2", H, c2w)
                Wo_sb = wtiles("Wo", H, c2w)
                Ud2_sb = wtiles("Ud2", H, c2w)

                th = {}
                prows = {}
                ps_stack = contextlib.ExitStack()
                ktp = ps_stack.enter_context(
                    tc.tile_pool(name="ktp", bufs=2, space="PSUM"))
                sp = ps_stack.enter_context(
                    tc.tile_pool(name="sp", bufs=1, space="PSUM"))
                pCps = ps_stack.enter_context(
                    tc.tile_pool(name="pCps", bufs=1, space="PSUM"))
                for i in range(bl + 2):
                    if i < bl:
                        b = i
                        # keys^T per at-chunk, tanh straight out of PSUM
                        for at in range(ACH):
                            kt = ktp.tile([P, t_dim], F32, tag="kt",
                                          name=f"kt{b}_{at}")
                            for h in range(2):
                                for dc in range(DCH):
                                    nc.tensor.matmul(
                                        kt[:, h * 512: (h + 1) * 512],
                                        Wk_sb[dc][:, at * P: (at + 1) * P],
                                        memT[(b, dc)][:, h * 512: (h + 1) * 512],
                                        start=(dc == 0), stop=(dc == DCH - 1))
                            tht = thp.tile([P, t_dim], BF16, tag="th",
                                           name=f"th{b}_{at}")
                            nc.scalar.activation(
                                tht[:], kt[:], AF.Tanh,
                                bias=qT[:, at * bl + b: at * bl + b + 1])
                            th[(b, at)] = tht
                    if 1 <= i <= bl:
                        b = i - 1
                        # s = v . th  (v stationary, PSUM row 0)
                        s_ps = sp.tile([1, t_dim], F32, tag="s", name=f"s{b}")
                        for h in range(2):
                            for at in range(ACH):
                                nc.tensor.matmul(
                                    s_ps[:, h * 512: (h + 1) * 512],
                                    v_sb[:, at: at + 1],
                                    th[(b, at)][:, h * 512: (h + 1) * 512],
                                    start=(at == 0), stop=(at == ACH - 1))
                        exprow = rows.tile([1, t_dim], BF16, tag="exprow",
                                           name=f"exprow{b}")
                        den = rows.tile([1, 1], F32, tag="den", name=f"den{b}")
                        nc.scalar.activation(exprow[:], s_ps[:], AF.Exp,
                                             accum_out=den[:])
                        inv = rows.tile([1, 1], F32, tag="inv", name=f"inv{b}")
                        nc.vector.reciprocal(inv[:], den[:])
                        prow = rows.tile([1, t_dim], BF16, tag="prow",
                                         name=f"prow{b}")
                        nc.vector.tensor_scalar_mul(prow[:], exprow[:], inv[:])
                        prows[b] = prow
                    if i >= 2:
                        b = i - 2
                        pbc = pCps.tile([P, t_dim], F32, tag="pbcC",
                                        name=f"pbcC{b}")
                        nc.tensor.matmul(pbc[:, 0:512], onesb[:],
                                         prows[b][:, 0:512], start=True, stop=True)
                        nc.tensor.matmul(pbc[:, 512:1024], onesb[:],
                                         prows[b][:, 512:1024], start=True,
                                         stop=True)
                        pb = pbp.tile([P, t_dim], BF16, tag="pb", name=f"pb{b}")
                        nc.scalar.copy(pb[:], pbc[:])
                        for dc in range(DCH):
                            scr = scrC.tile([P, t_dim], BF16, tag="scrC",
                                            name=f"scrC{b}_{dc}")
                            nc.vector.affine_mul_reduce(
                                out=scr[:],
                                accum_out=ctxF[:, dc * bl + b: dc * bl + b + 1],
                                in0=memT[(b, dc)][:], in1=pb[:],
                                scale=1.0, bias=0.0)
                nc.vector.tensor_copy(ctxT_bf[:], ctxF[:])
                ps_stack.close()

                if cfg.get("phases", 4) < 4:
                    nc.sync.dma_start(dr["out"][:, :4], ctxF[:bl, :4])
                    return

                # ================= CHAIN 2 =================
                with tc.tile_pool(name="cp2", bufs=3, space="PSUM") as cp2:
                    h1T = svp.tile([P, HCH * bl], BF16, tag="h1T", name="h1T")
                    h2T = svp.tile([P, HCH * bl], BF16, tag="h2T", name="h2T")

                    def gi_rhs_d1(kc):
                        if kc < ECH:
                            return attn_hT[:, kc * bl: (kc + 1) * bl]
                        return ctxT_bf[:, (kc - ECH) * bl: (kc - ECH + 1) * bl]

                    gru(cp2, HCH, Wd1_sb, Ud1_sb, gi_rhs_d1,
                        lambda kc: pd1T[:, kc * bl: (kc + 1) * bl],
                        bd1sT, bd1iT, bd1rT, pd1T, h1T)
                    gru(cp2, HCH, Wd2_sb, Ud2_sb,
                        lambda kc: h1T[:, kc * bl: (kc + 1) * bl],
                        lambda kc: pd2T[:, kc * bl: (kc + 1) * bl],
                        bd2sT, bd2iT, bd2rT, pd2T, h2T)

                    go = cp2.tile([P, OCH * bl], F32, tag="cps", name="go")
                    for mc in range(OCH):
                        for kc in range(HCH):
                            nc.tensor.matmul(
                                go[:, mc * bl: (mc + 1) * bl],
                                Wo_sb[kc][:, mc * P: (mc + 1) * P],
                                h2T[:, kc * bl: (kc + 1) * bl],
                                start=(kc == 0), stop=(kc == HCH - 1))
                    outT = svp.tile([P, OCH * bl], F32, tag="outT", name="outT")
                    for mc in range(OCH):
                        nc.scalar.activation(
                            outT[:, mc * bl: (mc + 1) * bl],
                            go[:, mc * bl: (mc + 1) * bl], AF.Identity,
                            bias=boT[:, mc: mc + 1])
                    for mc in range(OCH):
                        sz = min(P, OUTD - mc * P)
                        if sz <= 0:
                            break
                        nc.sync.dma_start(
                            dr["out"][:, mc * P: mc * P + sz].rearrange(
                                "b o -> o b"),
                            outT[:sz, mc * bl: (mc + 1) * bl])


def build(cfg=None):
    cfg = cfg or {"BL": BL, "T": T}
    nc = bacc.Bacc("TRN2", target_bir_lowering=False, debug=False,
                   num_devices=NCORES)
    bl, t_dim = cfg["BL"], cfg["T"]
    dr = {}

    def din(name, shape, dt=F32):
        dr[name] = nc.dram_tensor(name, list(shape), dt, kind="ExternalInput").ap()

    din("memT", [bl, DCH, P, t_dim], BF16)
    din("palr", [bl, t_dim], BF16)
    din("onesb", [1, P], BF16)
    din("v_sb", [P, ACH], BF16)
    din("xT", [P, OCH * bl], BF16)
    din("pahT", [P, ECH * bl], BF16)
    din("pd1T", [P, HCH * bl], BF16)
    din("pd2T", [P, HCH * bl], BF16)
    for nm, sh in [("bp1T", [P, ECH]), ("bp2T", [P, HCH]), ("baT", [P, ECH]),
                   ("bgsT", [P, 12]), ("bgiT", [P, 12]), ("bgrT", [P, 12]),
                   ("bd1sT", [P, 6]), ("bd1iT", [P, 6]), ("bd1rT", [P, 6]),
                   ("bd2sT", [P, 6]), ("bd2iT", [P, 6]), ("bd2rT", [P, 6]),
                   ("boT", [P, OCH])]:
        din(nm, sh)
    for nm, sh in [("Wp1", [KIN, E]), ("Wp2", [E, H]), ("Wa", [D, E]),
                   ("Wq", [E, A]), ("Wk", [D, A]), ("Wg", [H + E, 3 * E]),
                   ("Ug", [E, 3 * E]), ("Wd1", [E + D, 3 * H]),
                   ("Ud1", [H, 3 * H]), ("Wd2", [H, 3 * H]),
                   ("Ud2", [H, 3 * H]), ("Wo", [H, KOUT])]:
        din(nm, sh, BF16)
    dr["out"] = nc.dram_tensor("out", [bl, OUTD], F32, kind="ExternalOutput").ap()

    _emit(nc, dr, cfg)
    nc.compile()
    return nc


# ---------------- host-side data prep ----------------

def _chunkT(mat, pad_rows=None):
    """[b, F] -> transposed chunk layout [128, nch*b] (col = chunk*b + batch)."""
    a = np.asarray(mat, np.float32).T  # [F, b]
    f, b = a.shape
    if pad_rows and f < pad_rows:
        a = np.concatenate([a, np.zeros((pad_rows - f, b), np.float32)], 0)
    f = a.shape[0]
    nch = f // P
    return np.ascontiguousarray(
        a.reshape(nch, P, b).transpose(1, 0, 2).reshape(P, nch * b)
    )


def _biasT(vec, pad_to=None):
    a = np.asarray(vec, np.float32)
    if pad_to and a.shape[0] < pad_to:
        a = np.concatenate([a, np.zeros(pad_to - a.shape[0], np.float32)])
    nch = a.shape[0] // P
    return np.ascontiguousarray(a.reshape(nch, P).T)


def _prep_shared(inp):
    """Weights + constants shared by all cores."""
    bf = lambda x, pad=None: np.ascontiguousarray(
        (np.concatenate([np.asarray(x, np.float32),
                         np.zeros((pad[0] - x.shape[0], x.shape[1]), np.float32)], 0)
         if pad and x.shape[0] < pad[0] else
         np.concatenate([np.asarray(x, np.float32),
                         np.zeros((x.shape[0], pad[1] - x.shape[1]), np.float32)], 1)
         if pad and x.shape[1] < pad[1] else np.asarray(x, np.float32)
         ).astype(BF_NP))

    sh = {
        "onesb": np.ones((1, P), BF_NP),
        "v_sb": np.ascontiguousarray(
            np.asarray(inp["v_attn"], np.float32).reshape(ACH, P).T.astype(BF_NP)),
        "Wp1": bf(inp["Wp1"], pad=(KIN, E)),
        "Wp2": bf(inp["Wp2"]),
        "Wa": bf(inp["Wa"]),
        "Wq": bf(inp["Wq"]),
        "Wk": bf(inp["Wk"]),
        "Wg": bf(inp["Wg"]),
        "Ug": bf(inp["Ug"]),
        "Wd1": bf(inp["Wd1"]),
        "Ud1": bf(inp["Ud1"]),
        "Wd2": bf(inp["Wd2"]),
        "Ud2": bf(inp["Ud2"]),
        "Wo": bf(inp["Wo"], pad=(H, KOUT)),
        "bp1T": _biasT(inp["bp1"]),
        "bp2T": _biasT(inp["bp2"]),
        "baT": _biasT(inp["ba"]),
        "bgsT": _biasT(np.asarray(inp["bg_i"]) + np.asarray(inp["bg_r"])),
        "bgiT": _biasT(inp["bg_i"]),
        "bgrT": _biasT(inp["bg_r"]),
        "bd1sT": _biasT(np.asarray(inp["bd1_i"]) + np.asarray(inp["bd1_r"])),
        "bd1iT": _biasT(inp["bd1_i"]),
        "bd1rT": _biasT(inp["bd1_r"]),
        "bd2sT": _biasT(np.asarray(inp["bd2_i"]) + np.asarray(inp["bd2_r"])),
        "bd2iT": _biasT(inp["bd2_i"]),
        "bd2rT": _biasT(inp["bd2_r"]),
        "boT": _biasT(inp["bo"], pad_to=KOUT),
    }
    return sh


def _prep_core(inp, c, bl=BL, t_dim=T):
    sl = slice(c * bl, (c + 1) * bl)
    mem = np.asarray(inp["memory"], np.float32)[sl]        # [bl, t, D]
    memTv = np.ascontiguousarray(mem.transpose(0, 2, 1)).reshape(
        bl, DCH, P, t_dim).astype(BF_NP)
    return {
        "memT": memTv,
        "palr": np.asarray(inp["prev_alignments"], np.float32)[sl].astype(BF_NP),
        "xT": _chunkT(np.asarray(inp["inputs"], np.float32)[sl],
                      pad_rows=KIN).astype(BF_NP),
        "pahT": _chunkT(np.asarray(inp["prev_attn_h"], np.float32)[sl]).astype(BF_NP),
        "pd1T": _chunkT(np.asarray(inp["prev_dec_h1"], np.float32)[sl]).astype(BF_NP),
        "pd2T": _chunkT(np.asarray(inp["prev_dec_h2"], np.float32)[sl]).astype(BF_NP),
    }


_NC_CACHE = {}


def _get_nc():
    if "nc" not in _NC_CACHE:
        _NC_CACHE["nc"] = build()
    return _NC_CACHE["nc"]


def _run(inputs, **kw):
    nc = _get_nc()
    sh = _prep_shared(inputs)
    in_maps = [dict(sh, **_prep_core(inputs, c)) for c in range(NCORES)]
    res = run_bass_kernel_spmd(nc, in_maps, core_ids=list(range(NCORES)), **kw)
    out = np.concatenate([res.results[c]["out"] for c in range(NCORES)], 0)
    return out.reshape(B, 1, OUTD).astype(np.float32), res


def kernel(**inputs):
    out, _ = _run(inputs)
    return out


def _install_ntff_hook():
    """Register the axon NTFF profiling hook (missing antenv.axon_hooks)."""
    import contextlib
    import ctypes
    import types

    if "antenv.axon_hooks" in sys.modules:
        return
    lib = ctypes.CDLL("/opt/axon/libaxon_pjrt.so")
    if not hasattr(lib, "axon_start_nrt_profile"):
        return
    lib.axon_start_nrt_profile.argtypes = [
        ctypes.POINTER(ctypes.c_int64), ctypes.c_size_t]
    lib.axon_start_nrt_profile.restype = ctypes.c_int64
    lib.axon_stop_nrt_profile.argtypes = [ctypes.c_char_p]
    lib.axon_stop_nrt_profile.restype = ctypes.c_int64

    @contextlib.contextmanager
    def _hook(output_dir, device_ids):
        import jax

        jax.devices()
        if device_ids:
            ids = (ctypes.c_int64 * len(device_ids))(*device_ids)
            rc = lib.axon_start_nrt_profile(ids, len(device_ids))
        else:
            rc = lib.axon_start_nrt_profile(None, 0)
        if rc != 0:
            raise RuntimeError(f"axon_start_nrt_profile rc={rc}")
        try:
            yield
        finally:
            n = lib.axon_stop_nrt_profile(str(output_dir).encode())
            print(f"ntff profile: {n} file(s) written to {output_dir}")

    mod = types.ModuleType("antenv.axon_hooks")
    mod.get_axon_ntff_profile_hook = lambda: _hook
    mod.set_axon_ntff_profile_hook = lambda h: None
    sys.modules["antenv.axon_hooks"] = mod
    import antenv

    antenv.axon_hooks = mod


def kernel_traced(**inputs):
    """Dev helper: returns (output, BassKernelResults with exec_time_ns)."""
    _install_ntff_hook()
    return _run(inputs, trace=True)
